# revision 4
# baseline (speedup 1.0000x reference)
"""Trainium2 Bass kernel for iterative Hopfield update.

x <- softmax(x @ P^T) @ P, 3 iterations.
B=4096, N_PATTERNS=8192, N_NEURONS=1024, fp32.

Sharding: data-parallel over batch across 8 cores (512 rows each),
patterns replicated.

v2 vs the f32r baseline (779707 ns):
- Patterns (both blocked layouts) and E tiles in bf16: halves the HBM
  stream (the DMA queues were ~80% busy at f32) and enables the PE's
  fast-weight-load path (f32r stationary pays +11%/matmul on LDWEIGHTS:
  measured 236 ns vs the 213 ns ideal for 128x128x512).
- x stays f32r on-device (recurrent state keeps TF32-ish precision;
  moving operand rate is the same 1 cycle/row). Accuracy sim: bf16
  patterns+E with f32 x ~ few e-3 rel err, well under the 2e-2 gate;
  fp8 anywhere fails (ridge regime amplifies softmax-weight noise).
- The softmax denominator chain (partition-sum matmul -> broadcast
  matmul -> reciprocal) is emitted after phase-2's first m-tile group so
  its latency hides behind PE work instead of stalling between phases.
- acc is bitcast f32->f32r for the sum matmul instead of a DVE copy.
- j=0 pattern tile is loaded as 8 per-k-subtile DMAs so the first
  matmul can start ~2us into the kernel instead of ~20us.

Device-side layout (unchanged): everything transposed, XT = x^T [1024,512],
phase 1 computes S^T per 128-pattern tile j (K-accumulated over 8 neuron
tiles), exp via Act into bf16 E tiles, DVE accumulates E into f32 acc for
the denominators (cross-partition reduce via ones-column matmul); phase 2
computes O^T per 128-neuron tile m (K-accumulated over 64 pattern tiles)
and scales by the broadcast reciprocals.
"""

import numpy as np

B, P, N = 4096, 8192, 1024
N_CORES = 8
BLOC = B // N_CORES          # 512 batch rows per core
NJ = P // 128                # 64 pattern tiles
NK = N // 128                # 8 neuron tiles
N_ITER = 3
LOOP_REPS = 1

_cache = {}
_ONES = np.ones((128, 128), dtype=np.float32)


def _build():
    import concourse.bacc as bacc
    import concourse.tile as tile
    from concourse import mybir

    f32 = mybir.dt.float32
    f32r = mybir.dt.float32r
    bf16 = mybir.dt.bfloat16
    EXP = mybir.ActivationFunctionType.Exp

    nc = bacc.Bacc("TRN2", target_bir_lowering=False, debug=False)
    xt_d = nc.dram_tensor("xt", [N, BLOC], bf16, kind="ExternalInput").ap()
    ptb_d = nc.dram_tensor("ptb", [NJ, 128, NK * 128], bf16, kind="ExternalInput").ap()
    pb_d = nc.dram_tensor("pb", [NK, NJ // 8, 128, 8 * 128], bf16, kind="ExternalInput").ap()
    ones_d = nc.dram_tensor("ones", [128, 128], f32r, kind="ExternalInput").ap()
    ot_d = nc.dram_tensor("ot", [N, BLOC], f32r, kind="ExternalOutput").ap()

    with tile.TileContext(nc) as tc:
        with (
            tc.tile_pool(name="const", bufs=1) as const_pool,
            tc.tile_pool(name="xt", bufs=2) as xt_pool,
            tc.tile_pool(name="e", bufs=1) as e_pool,
            tc.tile_pool(name="pt", bufs=4) as pt_pool,
            tc.tile_pool(name="p2", bufs=4) as p2_pool,
            tc.tile_pool(name="misc", bufs=1) as misc_pool,
            tc.tile_pool(name="s_ps", bufs=4, space="PSUM") as s_ps_pool,
            tc.tile_pool(name="sum_ps", bufs=1, space="PSUM") as sum_ps_pool,
            tc.tile_pool(name="bc_ps", bufs=1, space="PSUM") as bc_ps_pool,
            tc.tile_pool(name="o_ps", bufs=2, space="PSUM") as o_ps_pool,
        ):
            # initial XT load (per k-tile so the first matmul starts early)
            xt_cur = []
            for k in range(NK):
                t = xt_pool.tile([128, BLOC], bf16, tag=f"xt{k}", name=f"xt{k}")
                for q in range(4):
                    nc.sync.dma_start(
                        t[32 * q:32 * (q + 1), :],
                        xt_d[128 * k + 32 * q:128 * k + 32 * (q + 1), :])
                xt_cur.append(t)
            # j=0 pattern tile, per-k-subtile DMAs
            pt0 = pt_pool.tile([128, NK * 128], bf16, tag="pt", name="pt0")
            for k in range(NK):
                nc.sync.dma_start(pt0[:, 128 * k:128 * (k + 1)],
                                  ptb_d[0, :, 128 * k:128 * (k + 1)])

            ones_col = const_pool.tile([128, 1], f32r, tag="ones_col", name="ones_col")
            nc.sync.dma_start(ones_col[:], ones_d[:, 0:1])
            ones_row = const_pool.tile([1, 128], f32r, tag="ones_row", name="ones_row")
            nc.sync.dma_start(ones_row[:], ones_d[0:1, :])

            for it in range(N_ITER):
                # ---- phase 1: S^T = P @ x^T per pattern tile, exp, sums ----
                e_tiles = []
                acc = misc_pool.tile([128, BLOC], f32r, tag="acc", name="acc")
                for j in range(NJ):
                    if it == 0 and j == 0:
                        pt_t = pt0
                    else:
                        pt_t = pt_pool.tile([128, NK * 128], bf16, tag="pt", name="ptj")
                        nc.sync.dma_start(pt_t[:], ptb_d[j])
                    s_ps = s_ps_pool.tile([128, BLOC], f32, tag="s", name="s_ps")
                    for k in range(NK):
                        nc.tensor.matmul(
                            s_ps[:],
                            pt_t[:, 128 * k:128 * (k + 1)],
                            xt_cur[k][:],
                            start=(k == 0),
                            stop=(k == NK - 1),
                        )
                    e_t = e_pool.tile([128, BLOC], bf16, tag=f"e{j}", name=f"e{j}")
                    nc.scalar.activation(e_t[:], s_ps[:], EXP)
                    e_tiles.append(e_t)
                    # softmax denominators: accumulate E on DVE (PE stays on matmuls)
                    if j == 0:
                        nc.vector.tensor_copy(acc[:], e_t[:])
                    else:
                        nc.vector.tensor_add(acc[:], acc[:], e_t[:])

                # ---- phase 2: O^T = sum_j P_block^T @ E[j], scale, next XT ----
                xt_next = []
                recip = None
                for m in range(NK):
                    o_ps = o_ps_pool.tile([128, BLOC], f32, tag="o", name="o_ps")
                    for kc in range(NJ // 8):
                        p2_t = p2_pool.tile([128, 8 * 128], bf16, tag="p2", name="p2")
                        nc.sync.dma_start(p2_t[:], pb_d[m, kc])
                        for g in range(8):
                            kk = 8 * kc + g
                            nc.tensor.matmul(
                                o_ps[:],
                                p2_t[:, 128 * g:128 * (g + 1)],
                                e_tiles[kk][:],
                                start=(kk == 0),
                                stop=(kk == NJ - 1),
                            )
                    if m == 0:
                        # denominator chain, emitted here so the PE work of
                        # m=0 hides its latency:
                        # cross-partition reduce via one ones-matmul
                        sum_ps = sum_ps_pool.tile([1, BLOC], f32, tag="sum", name="sum_ps")
                        nc.tensor.matmul(sum_ps[:], ones_col[:], acc[:],
                                         start=True, stop=True)
                        # denominators -> reciprocals broadcast to 128 partitions
                        sum_sb = misc_pool.tile([1, BLOC], f32r, tag="sum_sb", name="sum_sb")
                        nc.vector.tensor_copy(sum_sb[:], sum_ps[:])
                        bc_ps = bc_ps_pool.tile([128, BLOC], f32, tag="bc", name="bc_ps")
                        nc.tensor.matmul(bc_ps[:], ones_row[:], sum_sb[:],
                                         start=True, stop=True)
                        recip = misc_pool.tile([128, BLOC], f32, tag="recip", name="recip")
                        nc.vector.reciprocal(recip[:], bc_ps[:])
                    if it == N_ITER - 1:
                        xt_n = xt_pool.tile([128, BLOC], f32r, tag=f"xo{m}", name=f"xo{m}")
                        nc.vector.tensor_mul(xt_n[:], o_ps[:], recip[:])
                        for q in range(4):
                            nc.sync.dma_start(
                                ot_d[128 * m + 32 * q:128 * m + 32 * (q + 1), :],
                                xt_n[32 * q:32 * (q + 1), :])
                    else:
                        xt_n = xt_pool.tile([128, BLOC], bf16, tag=f"xt{m}", name=f"xtn{m}")
                        nc.vector.tensor_mul(xt_n[:], o_ps[:], recip[:])
                        xt_next.append(xt_n)
                xt_cur = xt_next

    nc.compile()
    return nc


def _prepare_inputs(x: np.ndarray, patterns: np.ndarray) -> list:
    import ml_dtypes

    x = np.ascontiguousarray(x, dtype=np.float32)
    patterns = np.ascontiguousarray(patterns, dtype=np.float32)

    # host-side tiling of the replicated patterns
    p4 = patterns.reshape(NJ, 128, NK, 128)          # [j, p, k, n]
    # ptb[j, n, k*128+p]: SBUF partition line n of block j, k-subtiles contiguous
    ptb = np.ascontiguousarray(p4.transpose(0, 3, 2, 1)).reshape(NJ, 128, NK * 128)
    # pb[m, kc, pat, g*128+n]: partition line pat, 8 k-subtiles contiguous
    pb = np.ascontiguousarray(
        p4.transpose(2, 0, 1, 3).reshape(NK, NJ // 8, 8, 128, 128)
          .transpose(0, 1, 3, 2, 4)
    ).reshape(NK, NJ // 8, 128, 8 * 128)
    ptb = ptb.astype(ml_dtypes.bfloat16)
    pb = pb.astype(ml_dtypes.bfloat16)
    xt = np.ascontiguousarray(x.T).astype(ml_dtypes.bfloat16)   # [N, B]
    return [
        {
            "xt": np.ascontiguousarray(xt[:, BLOC * i:BLOC * (i + 1)]),
            "ptb": ptb,
            "pb": pb,
            "ones": _ONES,
        }
        for i in range(N_CORES)
    ]


def kernel(x: np.ndarray, patterns: np.ndarray) -> np.ndarray:
    from concourse.bass_utils import run_bass_kernel_spmd

    if "nc" not in _cache:
        _cache["nc"] = _build()
    nc = _cache["nc"]

    in_maps = _prepare_inputs(x, patterns)
    res = run_bass_kernel_spmd(nc, in_maps, list(range(N_CORES))).results
    out = np.concatenate([res[i]["ot"].T for i in range(N_CORES)], axis=0)
    return np.ascontiguousarray(out.astype(np.float32))


# revision 5
# speedup vs baseline: 1.1136x; 1.1136x over previous
"""Trainium2 Bass kernel for iterative Hopfield update.

x <- softmax(x @ P^T) @ P, 3 iterations.
B=4096, N_PATTERNS=8192, N_NEURONS=1024, fp32.

Sharding: data-parallel over batch across 8 cores (512 rows each),
patterns replicated.

v3 design (from v1 f32r baseline 779707 ns -> v2 bf16 ~715-729 us):
- All matmul operands bf16 (patterns, E, x-state): 1 cycle/row like f32r
  but with fast-weight-load (216 ns/matmul measured vs f32r's 236-248)
  and half the HBM stream. Accuracy: all-bf16 = 1.04e-2 rel err on HW
  (gate 2e-2); fp8 anywhere except iter-3 phase-1 fails (the ridge
  regime amplifies softmax-weight noise ~|p|^2 per iteration, but by
  iter 3 |x| has collapsed 32 -> 0.107 so its score-quantization noise
  is negligible; sim 8.5e-3 -> 1.13e-2).
- Iter-3 phase-1 in fp8 e4m3 DoubleRow (K=256 per 216 ns pass, measured
  2x on this part): x2 is quantized to e4m3 scaled by 128 during
  iter-2's phase-2 DVE mul (reciprocal pre-scaled by 128), patterns
  scaled by 16 in a dedicated DR-paired layout, and the exp activation
  applies scale=1/2048 to undo both.
- DMA issue is a hidden serializer: each dma_start costs ~600 ns on the
  issuing engine's sequencer (observed as back-to-back DIRECT2D slices
  gating the first matmul at 30 us in v2). v3 issues from BOTH hwdge
  engines: Scalar(Act) takes xt head + p2 stream + output stores, Sync
  takes the pt stream; early pt tiles are split for lower latency.
- Softmax denominator chain (partition-sum matmul -> broadcast matmul
  -> DVE reciprocal) is emitted after phase-2's first m-tile group so
  it hides behind PE work. acc accumulates in f32r directly (the BIR
  verifier requires f32r-rounded producers for f32r matmul operands).

Layout: everything transposed. XT = x^T [1024, 512] as 8 [128, 512]
tiles (iter-3: 4 paired fp8 tiles [128, 2, 512]). Phase 1 per pattern
tile j: S^T[j] [128p, 512b] = sum_k PT-block^T XT_k, exp -> bf16 E[j];
DVE accumulates E into acc for denominators. Phase 2 per neuron tile m:
O^T[m] = sum_j P-block^T E[j], scaled by broadcast reciprocals.
"""

import numpy as np

B, P, N = 4096, 8192, 1024
N_CORES = 8
BLOC = B // N_CORES          # 512 batch rows per core
NJ = P // 128                # 64 pattern tiles
NK = N // 128                # 8 neuron tiles
N_ITER = 3
SX3 = 128.0                  # fp8 scale for x2 (elem std 0.009 -> ~1.15)
SP3 = 16.0                   # fp8 scale for patterns (elem std 0.1 -> 1.6)
USE_FP8_IT3 = True

_cache = {}
_ONES = np.ones((128, 128), dtype=np.float32)


def _build():
    import concourse.bacc as bacc
    import concourse.tile as tile
    from concourse import mybir

    f32 = mybir.dt.float32
    f32r = mybir.dt.float32r
    bf16 = mybir.dt.bfloat16
    fp8 = mybir.dt.float8e4
    EXP = mybir.ActivationFunctionType.Exp
    DR = mybir.MatmulPerfMode.DoubleRow

    nc = bacc.Bacc("TRN2", target_bir_lowering=False, debug=False)
    xt_d = nc.dram_tensor("xt", [N, BLOC], bf16, kind="ExternalInput").ap()
    ptb_d = nc.dram_tensor("ptb", [NJ, 128, NK * 128], bf16, kind="ExternalInput").ap()
    pb_d = nc.dram_tensor("pb", [NK, NJ // 8, 128, 8 * 128], bf16, kind="ExternalInput").ap()
    if USE_FP8_IT3:
        pt8_d = nc.dram_tensor("pt8", [NJ, 128, NK // 2, 2, 128], fp8,
                               kind="ExternalInput").ap()
    ones_d = nc.dram_tensor("ones", [128, 128], f32r, kind="ExternalInput").ap()
    ot_d = nc.dram_tensor("ot", [N, BLOC], f32r, kind="ExternalOutput").ap()

    with tile.TileContext(nc) as tc:
        with (
            tc.tile_pool(name="const", bufs=1) as const_pool,
            tc.tile_pool(name="xt", bufs=2) as xt_pool,
            tc.tile_pool(name="e", bufs=1) as e_pool,
            tc.tile_pool(name="pt", bufs=4) as pt_pool,
            tc.tile_pool(name="p2", bufs=4) as p2_pool,
            tc.tile_pool(name="misc", bufs=1) as misc_pool,
            tc.tile_pool(name="s_ps", bufs=4, space="PSUM") as s_ps_pool,
            tc.tile_pool(name="sum_ps", bufs=1, space="PSUM") as sum_ps_pool,
            tc.tile_pool(name="bc_ps", bufs=1, space="PSUM") as bc_ps_pool,
            tc.tile_pool(name="o_ps", bufs=2, space="PSUM") as o_ps_pool,
        ):
            # initial XT load, issued from Scalar (hwdge) while Sync does pt
            xt_cur = []
            for k in range(NK):
                t = xt_pool.tile([128, BLOC], bf16, tag=f"xt{k}", name=f"xt{k}")
                nc.scalar.dma_start(t[:], xt_d[128 * k:128 * (k + 1), :])
                xt_cur.append(t)
            # j=0 pattern tile: per-k-subtile DMAs on Sync (first matmul
            # needs only subtile 0)
            pt0 = pt_pool.tile([128, NK * 128], bf16, tag="pt", name="pt0")
            for k in range(NK):
                nc.sync.dma_start(pt0[:, 128 * k:128 * (k + 1)],
                                  ptb_d[0, :, 128 * k:128 * (k + 1)])
            # early pt tiles split in half to cut per-queue latency
            pt_early = {}
            for j in range(1, 5):
                t = pt_pool.tile([128, NK * 128], bf16, tag="pt", name=f"pte{j}")
                nc.sync.dma_start(t[:, :512], ptb_d[j, :, :512])
                nc.scalar.dma_start(t[:, 512:], ptb_d[j, :, 512:])
                pt_early[j] = t

            ones_col = const_pool.tile([128, 1], f32r, tag="ones_col", name="ones_col")
            nc.sync.dma_start(ones_col[:], ones_d[:, 0:1])
            ones_row = const_pool.tile([1, 128], f32r, tag="ones_row", name="ones_row")
            nc.sync.dma_start(ones_row[:], ones_d[0:1, :])

            for it in range(N_ITER):
                fp8_this = USE_FP8_IT3 and it == N_ITER - 1
                # ---- phase 1: scores + exp + denominator accumulation ----
                e_tiles = []
                acc = misc_pool.tile([128, BLOC], f32r, tag="acc", name="acc")
                for j in range(NJ):
                    if fp8_this:
                        pt8_t = pt_pool.tile([128, NK // 2, 2, 128], fp8,
                                             tag="pt8", name="pt8j")
                        nc.sync.dma_start(pt8_t[:], pt8_d[j])
                        s_ps = s_ps_pool.tile([128, BLOC], f32, tag="s", name="s_ps")
                        for kp in range(NK // 2):
                            nc.tensor.matmul(
                                s_ps[:],
                                pt8_t[:, kp],
                                xt_cur[kp][:],
                                start=(kp == 0),
                                stop=(kp == NK // 2 - 1),
                                perf_mode=DR,
                            )
                        e_t = e_pool.tile([128, BLOC], bf16, tag=f"e{j}", name=f"e{j}")
                        nc.scalar.activation(e_t[:], s_ps[:], EXP,
                                             scale=1.0 / (SX3 * SP3))
                    else:
                        if it == 0 and j == 0:
                            pt_t = pt0
                        elif it == 0 and j in pt_early:
                            pt_t = pt_early[j]
                        else:
                            pt_t = pt_pool.tile([128, NK * 128], bf16,
                                                tag="pt", name="ptj")
                            nc.sync.dma_start(pt_t[:], ptb_d[j])
                        s_ps = s_ps_pool.tile([128, BLOC], f32, tag="s", name="s_ps")
                        for k in range(NK):
                            nc.tensor.matmul(
                                s_ps[:],
                                pt_t[:, 128 * k:128 * (k + 1)],
                                xt_cur[k][:],
                                start=(k == 0),
                                stop=(k == NK - 1),
                            )
                        e_t = e_pool.tile([128, BLOC], bf16, tag=f"e{j}", name=f"e{j}")
                        nc.scalar.activation(e_t[:], s_ps[:], EXP)
                    e_tiles.append(e_t)
                    if j == 0:
                        nc.vector.tensor_copy(acc[:], e_t[:])
                    else:
                        nc.vector.tensor_add(acc[:], acc[:], e_t[:])

                # ---- phase 2: weighted pattern average, scale, next XT ----
                next_fp8 = USE_FP8_IT3 and it == N_ITER - 2
                xt_next = []
                xq_pairs = []
                recip = None
                for m in range(NK):
                    o_ps = o_ps_pool.tile([128, BLOC], f32, tag="o", name="o_ps")
                    for kc in range(NJ // 8):
                        p2_t = p2_pool.tile([128, 8 * 128], bf16, tag="p2", name="p2")
                        nc.scalar.dma_start(p2_t[:], pb_d[m, kc])
                        for g in range(8):
                            kk = 8 * kc + g
                            nc.tensor.matmul(
                                o_ps[:],
                                p2_t[:, 128 * g:128 * (g + 1)],
                                e_tiles[kk][:],
                                start=(kk == 0),
                                stop=(kk == NJ - 1),
                            )
                    if m == 0:
                        # denominator chain; hidden behind m=0's matmuls
                        sum_ps = sum_ps_pool.tile([1, BLOC], f32, tag="sum",
                                                  name="sum_ps")
                        nc.tensor.matmul(sum_ps[:], ones_col[:], acc[:],
                                         start=True, stop=True)
                        sum_sb = misc_pool.tile([1, BLOC], f32r, tag="sum_sb",
                                                name="sum_sb")
                        nc.vector.tensor_copy(sum_sb[:], sum_ps[:])
                        bc_ps = bc_ps_pool.tile([128, BLOC], f32, tag="bc",
                                                name="bc_ps")
                        nc.tensor.matmul(bc_ps[:], ones_row[:], sum_sb[:],
                                         start=True, stop=True)
                        recip = misc_pool.tile([128, BLOC], f32, tag="recip",
                                               name="recip")
                        nc.vector.reciprocal(recip[:], bc_ps[:])
                        if next_fp8:
                            # pre-scale reciprocals so the e4m3 x-state is
                            # well inside normal range
                            recip_s = misc_pool.tile([128, BLOC], f32,
                                                     tag="recip_s", name="recip_s")
                            nc.vector.tensor_scalar_mul(recip_s[:], recip[:], SX3)
                            recip = recip_s
                    if it == N_ITER - 1:
                        xt_n = xt_pool.tile([128, BLOC], f32r, tag=f"xo{m}",
                                            name=f"xo{m}")
                        nc.vector.tensor_mul(xt_n[:], o_ps[:], recip[:])
                        for q in range(4):
                            nc.scalar.dma_start(
                                ot_d[128 * m + 32 * q:128 * m + 32 * (q + 1), :],
                                xt_n[32 * q:32 * (q + 1), :])
                    elif next_fp8:
                        if m % 2 == 0:
                            xq = xt_pool.tile([128, 2, BLOC], fp8,
                                              tag=f"xq{m // 2}", name=f"xq{m // 2}")
                            xq_pairs.append(xq)
                        nc.vector.tensor_mul(xq_pairs[m // 2][:, m % 2, :],
                                             o_ps[:], recip[:])
                    else:
                        xt_n = xt_pool.tile([128, BLOC], bf16, tag=f"xt{m}",
                                            name=f"xtn{m}")
                        nc.vector.tensor_mul(xt_n[:], o_ps[:], recip[:])
                        xt_next.append(xt_n)
                xt_cur = xq_pairs if next_fp8 else xt_next

    nc.compile()
    return nc


def _prepare_inputs(x: np.ndarray, patterns: np.ndarray) -> list:
    import ml_dtypes

    x = np.ascontiguousarray(x, dtype=np.float32)
    patterns = np.ascontiguousarray(patterns, dtype=np.float32)

    # host-side tiling of the replicated patterns
    p4 = patterns.reshape(NJ, 128, NK, 128)          # [j, p, k, n]
    # ptb[j, n, k*128+p]: SBUF partition line n of block j, k-subtiles contiguous
    ptb = np.ascontiguousarray(p4.transpose(0, 3, 2, 1)).reshape(NJ, 128, NK * 128)
    # pb[m, kc, pat, g*128+n]: partition line pat, 8 k-subtiles contiguous
    pb = np.ascontiguousarray(
        p4.transpose(2, 0, 1, 3).reshape(NK, NJ // 8, 8, 128, 128)
          .transpose(0, 1, 3, 2, 4)
    ).reshape(NK, NJ // 8, 128, 8 * 128)
    ptb = ptb.astype(ml_dtypes.bfloat16)
    pb = pb.astype(ml_dtypes.bfloat16)
    xt = np.ascontiguousarray(x.T).astype(ml_dtypes.bfloat16)   # [N, B]
    in_map_common = {"ptb": ptb, "pb": pb, "ones": _ONES}
    if USE_FP8_IT3:
        # pt8[j, n, kp, i, m] = SP3 * P[j*128+m, (2kp+i)*128+n], e4m3 DR pairs
        p5 = (SP3 * p4).reshape(NJ, 128, NK // 2, 2, 128)   # [j, p, kp, i, n]
        pt8 = np.ascontiguousarray(p5.transpose(0, 4, 2, 3, 1)).astype(
            ml_dtypes.float8_e4m3)
        in_map_common["pt8"] = pt8
    return [
        dict(in_map_common,
             xt=np.ascontiguousarray(xt[:, BLOC * i:BLOC * (i + 1)]))
        for i in range(N_CORES)
    ]


def kernel(x: np.ndarray, patterns: np.ndarray) -> np.ndarray:
    from concourse.bass_utils import run_bass_kernel_spmd

    if "nc" not in _cache:
        _cache["nc"] = _build()
    nc = _cache["nc"]

    in_maps = _prepare_inputs(x, patterns)
    res = run_bass_kernel_spmd(nc, in_maps, list(range(N_CORES))).results
    out = np.concatenate([res[i]["ot"].T for i in range(N_CORES)], axis=0)
    return np.ascontiguousarray(out.astype(np.float32))


# revision 6
# speedup vs baseline: 1.1335x; 1.0179x over previous
"""Trainium2 Bass kernel for iterative Hopfield update.

x <- softmax(x @ P^T) @ P, 3 iterations.
B=4096, N_PATTERNS=8192, N_NEURONS=1024, fp32.

Sharding: data-parallel over batch across 8 cores (512 rows each),
patterns replicated.

v3 design (from v1 f32r baseline 779707 ns -> v2 bf16 ~715-729 us):
- All matmul operands bf16 (patterns, E, x-state): 1 cycle/row like f32r
  but with fast-weight-load (216 ns/matmul measured vs f32r's 236-248)
  and half the HBM stream. Accuracy: all-bf16 = 1.04e-2 rel err on HW
  (gate 2e-2); fp8 anywhere except iter-3 phase-1 fails (the ridge
  regime amplifies softmax-weight noise ~|p|^2 per iteration, but by
  iter 3 |x| has collapsed 32 -> 0.107 so its score-quantization noise
  is negligible; sim 8.5e-3 -> 1.13e-2).
- Iter-3 phase-1 in fp8 e4m3 DoubleRow (K=256 per 216 ns pass, measured
  2x on this part): x2 is quantized to e4m3 scaled by 128 during
  iter-2's phase-2 DVE mul (reciprocal pre-scaled by 128), patterns
  scaled by 16 in a dedicated DR-paired layout, and the exp activation
  applies scale=1/2048 to undo both.
- DMA issue is a hidden serializer: each dma_start costs ~600 ns on the
  issuing engine's sequencer (observed as back-to-back DIRECT2D slices
  gating the first matmul at 30 us in v2). v3 issues from BOTH hwdge
  engines: Scalar(Act) takes xt head + p2 stream + output stores, Sync
  takes the pt stream; early pt tiles are split for lower latency.
- Softmax denominator chain (partition-sum matmul -> broadcast matmul
  -> DVE reciprocal) is emitted after phase-2's first m-tile group so
  it hides behind PE work. acc accumulates in f32r directly (the BIR
  verifier requires f32r-rounded producers for f32r matmul operands).

Layout: everything transposed. XT = x^T [1024, 512] as 8 [128, 512]
tiles (iter-3: 4 paired fp8 tiles [128, 2, 512]). Phase 1 per pattern
tile j: S^T[j] [128p, 512b] = sum_k PT-block^T XT_k, exp -> bf16 E[j];
DVE accumulates E into acc for denominators. Phase 2 per neuron tile m:
O^T[m] = sum_j P-block^T E[j], scaled by broadcast reciprocals.
"""

import numpy as np

B, P, N = 4096, 8192, 1024
N_CORES = 8
BLOC = B // N_CORES          # 512 batch rows per core
NJ = P // 128                # 64 pattern tiles
NK = N // 128                # 8 neuron tiles
N_ITER = 3
SX3 = 128.0                  # fp8 scale for x2 (elem std 0.009 -> ~1.15)
SP3 = 16.0                   # fp8 scale for patterns (elem std 0.1 -> 1.6)
USE_FP8_IT3 = True

_cache = {}
_ONES = np.ones((128, 128), dtype=np.float32)


def _build():
    import concourse.bacc as bacc
    import concourse.tile as tile
    from concourse import mybir

    f32 = mybir.dt.float32
    f32r = mybir.dt.float32r
    bf16 = mybir.dt.bfloat16
    fp8 = mybir.dt.float8e4
    EXP = mybir.ActivationFunctionType.Exp
    DR = mybir.MatmulPerfMode.DoubleRow

    nc = bacc.Bacc("TRN2", target_bir_lowering=False, debug=False)
    xt_d = nc.dram_tensor("xt", [N, BLOC], bf16, kind="ExternalInput").ap()
    ptb_d = nc.dram_tensor("ptb", [NJ, 128, NK * 128], bf16, kind="ExternalInput").ap()
    pb_d = nc.dram_tensor("pb", [NK, NJ // 8, 128, 8 * 128], bf16, kind="ExternalInput").ap()
    if USE_FP8_IT3:
        pt8_d = nc.dram_tensor("pt8", [NJ, 128, NK // 2, 2, 128], fp8,
                               kind="ExternalInput").ap()
    ones_d = nc.dram_tensor("ones", [128, 128], f32r, kind="ExternalInput").ap()
    ot_d = nc.dram_tensor("ot", [N, BLOC], f32r, kind="ExternalOutput").ap()

    with tile.TileContext(nc) as tc:
        with (
            tc.tile_pool(name="const", bufs=1) as const_pool,
            tc.tile_pool(name="xt", bufs=2) as xt_pool,
            tc.tile_pool(name="e", bufs=1) as e_pool,
            tc.tile_pool(name="pt", bufs=4) as pt_pool,
            tc.tile_pool(name="pt8", bufs=10) as pt8_pool,
            tc.tile_pool(name="p2pre", bufs=1) as p2pre_pool,
            tc.tile_pool(name="p2", bufs=4) as p2_pool,
            tc.tile_pool(name="misc", bufs=1) as misc_pool,
            tc.tile_pool(name="s_ps", bufs=4, space="PSUM") as s_ps_pool,
            tc.tile_pool(name="sum_ps", bufs=1, space="PSUM") as sum_ps_pool,
            tc.tile_pool(name="bc_ps", bufs=1, space="PSUM") as bc_ps_pool,
            tc.tile_pool(name="o_ps", bufs=2, space="PSUM") as o_ps_pool,
        ):
            # initial XT load, issued from Scalar (hwdge) while Sync does pt
            xt_cur = []
            for k in range(NK):
                t = xt_pool.tile([128, BLOC], bf16, tag=f"xt{k}", name=f"xt{k}")
                nc.scalar.dma_start(t[:], xt_d[128 * k:128 * (k + 1), :])
                xt_cur.append(t)
            # j=0 pattern tile: per-k-subtile DMAs on Sync (first matmul
            # needs only subtile 0)
            pt0 = pt_pool.tile([128, NK * 128], bf16, tag="pt", name="pt0")
            for k in range(NK):
                nc.sync.dma_start(pt0[:, 128 * k:128 * (k + 1)],
                                  ptb_d[0, :, 128 * k:128 * (k + 1)])
            # early pt tiles split in half to cut per-queue latency
            pt_early = {}
            for j in range(1, 9):
                t = pt_pool.tile([128, NK * 128], bf16, tag="pt", name=f"pte{j}")
                nc.sync.dma_start(t[:, :512], ptb_d[j, :, :512])
                nc.scalar.dma_start(t[:, 512:], ptb_d[j, :, 512:])
                pt_early[j] = t

            ones_col = const_pool.tile([128, 1], f32r, tag="ones_col", name="ones_col")
            nc.sync.dma_start(ones_col[:], ones_d[:, 0:1])
            ones_row = const_pool.tile([1, 128], f32r, tag="ones_row", name="ones_row")
            nc.sync.dma_start(ones_row[:], ones_d[0:1, :])

            for it in range(N_ITER):
                fp8_this = USE_FP8_IT3 and it == N_ITER - 1
                # prefetch phase-2 m=0's pattern row now; consumed ~a full
                # phase later, so the transfers are long done by then
                p2pre = []
                for kc in range(NJ // 8):
                    t = p2pre_pool.tile([128, 8 * 128], bf16, tag=f"p2pre{kc}",
                                        name=f"p2pre{kc}")
                    nc.sync.dma_start(t[:], pb_d[0, kc])
                    p2pre.append(t)
                # ---- phase 1: scores + exp + denominator accumulation ----
                e_tiles = []
                acc = misc_pool.tile([128, BLOC], f32r, tag="acc", name="acc")
                for j in range(NJ):
                    if fp8_this:
                        pt8_t = pt8_pool.tile([128, NK // 2, 2, 128], fp8,
                                              tag="pt8", name="pt8j")
                        nc.sync.dma_start(pt8_t[:], pt8_d[j])
                        s_ps = s_ps_pool.tile([128, BLOC], f32, tag="s", name="s_ps")
                        for kp in range(NK // 2):
                            nc.tensor.matmul(
                                s_ps[:],
                                pt8_t[:, kp],
                                xt_cur[kp][:],
                                start=(kp == 0),
                                stop=(kp == NK // 2 - 1),
                                perf_mode=DR,
                            )
                        e_t = e_pool.tile([128, BLOC], bf16, tag=f"e{j}", name=f"e{j}")
                        nc.scalar.activation(e_t[:], s_ps[:], EXP,
                                             scale=1.0 / (SX3 * SP3))
                    else:
                        if it == 0 and j == 0:
                            pt_t = pt0
                        elif it == 0 and j in pt_early:
                            pt_t = pt_early[j]
                        else:
                            pt_t = pt_pool.tile([128, NK * 128], bf16,
                                                tag="pt", name="ptj")
                            nc.sync.dma_start(pt_t[:], ptb_d[j])
                        s_ps = s_ps_pool.tile([128, BLOC], f32, tag="s", name="s_ps")
                        for k in range(NK):
                            nc.tensor.matmul(
                                s_ps[:],
                                pt_t[:, 128 * k:128 * (k + 1)],
                                xt_cur[k][:],
                                start=(k == 0),
                                stop=(k == NK - 1),
                            )
                        e_t = e_pool.tile([128, BLOC], bf16, tag=f"e{j}", name=f"e{j}")
                        nc.scalar.activation(e_t[:], s_ps[:], EXP)
                    e_tiles.append(e_t)
                    if j == 0:
                        nc.vector.tensor_copy(acc[:], e_t[:])
                    else:
                        nc.vector.tensor_add(acc[:], acc[:], e_t[:])

                # ---- phase 2: weighted pattern average, scale, next XT ----
                next_fp8 = USE_FP8_IT3 and it == N_ITER - 2
                xt_next = []
                xq_pairs = []
                recip = None
                for m in range(NK):
                    o_ps = o_ps_pool.tile([128, BLOC], f32, tag="o", name="o_ps")
                    for kc in range(NJ // 8):
                        if m == 0:
                            p2_t = p2pre[kc]
                        else:
                            p2_t = p2_pool.tile([128, 8 * 128], bf16, tag="p2",
                                                name="p2")
                            nc.scalar.dma_start(p2_t[:], pb_d[m, kc])
                        for g in range(8):
                            kk = 8 * kc + g
                            nc.tensor.matmul(
                                o_ps[:],
                                p2_t[:, 128 * g:128 * (g + 1)],
                                e_tiles[kk][:],
                                start=(kk == 0),
                                stop=(kk == NJ - 1),
                            )
                    if m == 0:
                        # denominator chain; hidden behind m=0's matmuls
                        sum_ps = sum_ps_pool.tile([1, BLOC], f32, tag="sum",
                                                  name="sum_ps")
                        nc.tensor.matmul(sum_ps[:], ones_col[:], acc[:],
                                         start=True, stop=True)
                        sum_sb = misc_pool.tile([1, BLOC], f32r, tag="sum_sb",
                                                name="sum_sb")
                        nc.vector.tensor_copy(sum_sb[:], sum_ps[:])
                        bc_ps = bc_ps_pool.tile([128, BLOC], f32, tag="bc",
                                                name="bc_ps")
                        nc.tensor.matmul(bc_ps[:], ones_row[:], sum_sb[:],
                                         start=True, stop=True)
                        recip = misc_pool.tile([128, BLOC], f32, tag="recip",
                                               name="recip")
                        nc.vector.reciprocal(recip[:], bc_ps[:])
                        if next_fp8:
                            # pre-scale reciprocals so the e4m3 x-state is
                            # well inside normal range
                            recip_s = misc_pool.tile([128, BLOC], f32,
                                                     tag="recip_s", name="recip_s")
                            nc.vector.tensor_scalar_mul(recip_s[:], recip[:], SX3)
                            recip = recip_s
                    if it == N_ITER - 1:
                        xt_n = xt_pool.tile([128, BLOC], f32r, tag=f"xo{m}",
                                            name=f"xo{m}")
                        nc.vector.tensor_mul(xt_n[:], o_ps[:], recip[:])
                        for q in range(4):
                            nc.sync.dma_start(
                                ot_d[128 * m + 32 * q:128 * m + 32 * (q + 1), :],
                                xt_n[32 * q:32 * (q + 1), :])
                    elif next_fp8:
                        if m % 2 == 0:
                            xq = xt_pool.tile([128, 2, BLOC], fp8,
                                              tag=f"xq{m // 2}", name=f"xq{m // 2}")
                            xq_pairs.append(xq)
                        nc.vector.tensor_mul(xq_pairs[m // 2][:, m % 2, :],
                                             o_ps[:], recip[:])
                    else:
                        xt_n = xt_pool.tile([128, BLOC], bf16, tag=f"xt{m}",
                                            name=f"xtn{m}")
                        nc.vector.tensor_mul(xt_n[:], o_ps[:], recip[:])
                        xt_next.append(xt_n)
                xt_cur = xq_pairs if next_fp8 else xt_next

    nc.compile()
    return nc


def _prepare_inputs(x: np.ndarray, patterns: np.ndarray) -> list:
    import ml_dtypes

    x = np.ascontiguousarray(x, dtype=np.float32)
    patterns = np.ascontiguousarray(patterns, dtype=np.float32)

    # host-side tiling of the replicated patterns
    p4 = patterns.reshape(NJ, 128, NK, 128)          # [j, p, k, n]
    # ptb[j, n, k*128+p]: SBUF partition line n of block j, k-subtiles contiguous
    ptb = np.ascontiguousarray(p4.transpose(0, 3, 2, 1)).reshape(NJ, 128, NK * 128)
    # pb[m, kc, pat, g*128+n]: partition line pat, 8 k-subtiles contiguous
    pb = np.ascontiguousarray(
        p4.transpose(2, 0, 1, 3).reshape(NK, NJ // 8, 8, 128, 128)
          .transpose(0, 1, 3, 2, 4)
    ).reshape(NK, NJ // 8, 128, 8 * 128)
    ptb = ptb.astype(ml_dtypes.bfloat16)
    pb = pb.astype(ml_dtypes.bfloat16)
    xt = np.ascontiguousarray(x.T).astype(ml_dtypes.bfloat16)   # [N, B]
    in_map_common = {"ptb": ptb, "pb": pb, "ones": _ONES}
    if USE_FP8_IT3:
        # pt8[j, n, kp, i, m] = SP3 * P[j*128+m, (2kp+i)*128+n], e4m3 DR pairs
        p5 = (SP3 * p4).reshape(NJ, 128, NK // 2, 2, 128)   # [j, p, kp, i, n]
        pt8 = np.ascontiguousarray(p5.transpose(0, 4, 2, 3, 1)).astype(
            ml_dtypes.float8_e4m3)
        in_map_common["pt8"] = pt8
    return [
        dict(in_map_common,
             xt=np.ascontiguousarray(xt[:, BLOC * i:BLOC * (i + 1)]))
        for i in range(N_CORES)
    ]


def kernel(x: np.ndarray, patterns: np.ndarray) -> np.ndarray:
    from concourse.bass_utils import run_bass_kernel_spmd

    if "nc" not in _cache:
        _cache["nc"] = _build()
    nc = _cache["nc"]

    in_maps = _prepare_inputs(x, patterns)
    res = run_bass_kernel_spmd(nc, in_maps, list(range(N_CORES))).results
    out = np.concatenate([res[i]["ot"].T for i in range(N_CORES)], axis=0)
    return np.ascontiguousarray(out.astype(np.float32))
